# revision 12
# baseline (speedup 1.0000x reference)
"""Trainium2 Bass kernel for nn_ChannelBlock (XCiT-style channel-attention block).

Sharding: data-parallel over batch (32 images -> 8 cores x 4 images). Each core
runs the full block on its 4 images. Matmuls run in fp32r (full-rate, ~11
mantissa bits input rounding, fp32 accumulate); the tiny attention einsums and
the depthwise-conv taps run in bf16 (their contribution to the output is small).

Structure per image (channel-major [C_part, N_free] activations):
  conv1   9 bf16 diagonal-matmul taps w/ shifted boundary-clipped APs + f32r
          bias tap into PSUM; residual added in the PSUM drain (stt on DVE).
  LN1     sums via ones-matmuls into PSUM; 1/sqrt(var+eps) = exp(-.5*ln()) on
          ACT; s replicated across partitions by a K=1 ones matmul; gain g
          folded into the qkv weights host-side; mean/bias applied as K=2
          rank corrections {ms=mu*s, ones} x {-rowsum(W'), W@b} in PSUM.
  attn    q channel-major; kT/vT token-major (so k^T v contracts on the
          partition dim), both bf16; einsum1 computed in both orientations
          ([d,e] for row sums via exp's fused accum_out + additive -30 mask
          off the head-diagonal; [e,d] as einsum2's stationary operand);
          softmax 1/Z folded into the einsum2 PSUM drain.
  proj    f32r matmul; bias+residual in the drain (stt).
  conv2 / LN2 / MLP (fc1+exact gelu+fc2) same tricks; fc2 drain adds bias and
          the residual, then DMA out.
"""
import os
import sys
sys.path.insert(0, "/opt/trn_rl_repo")

import numpy as np
import ml_dtypes

import concourse.bass as bass
import concourse.mybir as mybir
import concourse.tile as tile

F32 = mybir.dt.float32
F32R = mybir.dt.float32r
BF16 = mybir.dt.bfloat16
AF = mybir.ActivationFunctionType
ALU = mybir.AluOpType

NCORES = 8
BLOC = 4
C = 384
CB = 3
HWD = 28
N = 784
NH = 392
HEADS = 12
HD = 32
HIDDEN = 1536
HB = 12
SCALE = HD ** -0.5
EPS = 1e-5
CHUNKS = [(0, 128), (128, 128), (256, 128), (384, 128),
          (512, 128), (640, 128), (768, 16)]
TAPS = [(0, 0)] + [(dy, dx) for dy in (-1, 0, 1) for dx in (-1, 0, 1)
                   if (dy, dx) != (0, 0)]
PADW = 816  # pitch-29 padded conv-input row layout: 1 leading pad + 28x29 + tail


def _legalize_multiwaits(nc):
    """This walrus build accepts at most ONE semaphore wait per instruction.
    Tile keeps almost everything at <=1 but the kernel-tail drain (and the
    fused fp32r matmuls) can carry more; splice NoOp carriers in front."""
    n = 0
    for fn in nc.m.functions:
        for blk in fn.blocks:
            new_list = []
            changed = False
            for inst in blk.instructions:
                si = inst.sync_info
                if si is not None and si.on_wait is not None and len(si.on_wait) > 1:
                    waits = list(si.on_wait)
                    for i, w in enumerate(waits[:-1]):
                        new_list.append(mybir.InstNoOp(
                            name=f"{inst.name}_waitcarrier{i}",
                            opcode="NoOp",
                            engine=inst.engine,
                            sync_info=mybir.SyncInfo(on_wait=[w], on_update=[]),
                        ))
                        n += 1
                    si.on_wait = waits[-1:]
                    changed = True
                new_list.append(inst)
            if changed:
                blk.instructions[:] = new_list
    return n


def _h2(ap):
    return ap.rearrange("p (h n) -> p h n", h=2)


def _h2rc(ap):
    return ap.rearrange("p (h r c) -> p h r c", h=2, c=HWD)


def _rc(ap):
    return ap.rearrange("p (r c) -> p r c", c=HWD)


def build_program(legalize=True):
    nc = bass.Bass("TRN2", target_bir_lowering=False)

    x_d = nc.dram_tensor("x", [BLOC, C, N], F32, kind="ExternalInput")
    xb_d = nc.dram_tensor("xb", [BLOC, C, N], BF16, kind="ExternalInput")
    wqkv_d = nc.dram_tensor("wqkv", [C, 3 * C], F32, kind="ExternalInput")
    wproj_d = nc.dram_tensor("wproj", [C, C], F32, kind="ExternalInput")
    wfc1_d = nc.dram_tensor("wfc1", [C, HIDDEN], F32, kind="ExternalInput")
    wfc2_d = nc.dram_tensor("wfc2", [HIDDEN, C], F32, kind="ExternalInput")
    diag_d = nc.dram_tensor("diag", [2, 9, CB, 128, 128], BF16, kind="ExternalInput")
    rkq_d = nc.dram_tensor("rkq", [2, C], F32, kind="ExternalInput")
    rkkv_d = nc.dram_tensor("rkkv", [2, 2 * C], F32, kind="ExternalInput")
    rkf_d = nc.dram_tensor("rkf", [2, HIDDEN], F32, kind="ExternalInput")
    cpeb_d = nc.dram_tensor("cpeb", [1, 2 * C], F32, kind="ExternalInput")
    onesrow_d = nc.dram_tensor("onesrow", [1, N], F32, kind="ExternalInput")
    onesp_d = nc.dram_tensor("onesp", [128, 2], F32, kind="ExternalInput")
    onespb_d = nc.dram_tensor("onespb", [128, 1], BF16, kind="ExternalInput")
    pbias_d = nc.dram_tensor("pbias", [128, 6], F32, kind="ExternalInput")
    maskl_d = nc.dram_tensor("maskl", [5, 128], BF16, kind="ExternalInput")
    maskr_d = nc.dram_tensor("maskr", [5, 128], BF16, kind="ExternalInput")
    out_d = nc.dram_tensor("out", [BLOC, C, N], F32, kind="ExternalOutput")

    with tile.TileContext(nc) as tc:
        with tc.tile_pool(name="const", bufs=1) as cp, \
             tc.tile_pool(name="act1", bufs=1) as a1, \
             tc.tile_pool(name="act3", bufs=3) as a3, \
             tc.tile_pool(name="psw", bufs=2, space="PSUM") as pw, \
             tc.tile_pool(name="psn", bufs=4, space="PSUM") as pn:

            # ---------------- constants ----------------
            wqkv = cp.tile([128, CB, 3 * C], F32R, tag="wqkv")
            wproj = cp.tile([128, CB, C], F32R, tag="wproj")
            wfc1 = cp.tile([128, CB, HIDDEN], F32R, tag="wfc1")
            wfc2 = cp.tile([128, HB, C], F32R, tag="wfc2")
            for cb in range(CB):
                nc.sync.dma_start(wqkv[:, cb, :],
                                  wqkv_d[cb * 128:(cb + 1) * 128, :].bitcast(F32R))
                nc.sync.dma_start(wproj[:, cb, :],
                                  wproj_d[cb * 128:(cb + 1) * 128, :].bitcast(F32R))
                nc.sync.dma_start(wfc1[:, cb, :],
                                  wfc1_d[cb * 128:(cb + 1) * 128, :].bitcast(F32R))
            for j in range(HB):
                nc.sync.dma_start(wfc2[:, j, :],
                                  wfc2_d[j * 128:(j + 1) * 128, :].bitcast(F32R))
            diag = cp.tile([128, 2, 9, CB, 128], BF16, tag="diag")
            for cv in range(2):
                for cb in range(CB):
                    nc.sync.dma_start(
                        diag[:, cv, :, cb, :],
                        diag_d[cv, :, cb, :, :].rearrange("t p f -> p t f"))
            rkq = cp.tile([2, C], F32R, tag="rkq")
            nc.sync.dma_start(rkq[:], rkq_d[:].bitcast(F32R))
            rkkv = cp.tile([2, 2 * C], F32R, tag="rkkv")
            nc.sync.dma_start(rkkv[:], rkkv_d[:].bitcast(F32R))
            rkf = cp.tile([2, HIDDEN], F32R, tag="rkf")
            nc.sync.dma_start(rkf[:], rkf_d[:].bitcast(F32R))
            cpeb = cp.tile([1, 2 * C], F32R, tag="cpeb")
            nc.sync.dma_start(cpeb[:], cpeb_d[:].bitcast(F32R))
            onesrow = cp.tile([1, N], F32R, tag="onesrow")
            nc.sync.dma_start(onesrow[:], onesrow_d[:].bitcast(F32R))
            onesp = cp.tile([128, 2], F32R, tag="onesp")
            nc.sync.dma_start(onesp[:], onesp_d[:].bitcast(F32R))
            onespb = cp.tile([128, 1], BF16, tag="onespb")
            nc.sync.dma_start(onespb[:], onespb_d[:])
            pbias = cp.tile([128, 6], F32, tag="pbias")
            nc.sync.dma_start(pbias[:], pbias_d[:])
            maskl = cp.tile([5, 128], BF16, tag="maskl")
            nc.sync.dma_start(maskl[:], maskl_d[:])
            maskr = cp.tile([5, 128], BF16, tag="maskr")
            nc.sync.dma_start(maskr[:], maskr_d[:])

            # ---------------- helpers ----------------
            def conv_core(srcp, res_src, cv, tag):
                """srcp: bf16 [128, CB, PADW] pitch-29 padded tap source (1
                leading pad element, 1 pad col per row, all pads zero);
                res_src: f32r residual source.
                Returns out = res_src + dwconv(src) + b, f32r.

                Every tap writes FULL-WIDTH flat rows into PSUM; horizontal
                shifts read the zero pad column at row wraps, vertical edge
                rows are clipped."""
                out = a1.tile([128, CB, N], F32R, tag=tag)
                for cb in range(CB):
                    pc = pw.tile([128, 2, 512], F32, tag="pw")
                    for ti, (dy, dx) in enumerate(TAPS):
                        lhsT = diag[:, cv, ti, cb, :]
                        r0 = max(0, -dy)
                        r1 = HWD - max(0, dy)
                        for h in range(2):
                            rA = max(r0, 14 * h)
                            rB = min(r1, 14 * h + 14)
                            if rA >= rB:
                                continue
                            rl0, rl1 = rA - 14 * h, rB - 14 * h
                            outap = pc[:, h, rl0 * HWD:rl1 * HWD]
                            s0 = 1 + (rA + dy) * 29 + dx
                            inap = srcp[:, cb, s0:s0 + (rB - rA) * 29].rearrange(
                                "p (r c) -> p r c", c=29)[:, :, 0:HWD]
                            nc.tensor.matmul(outap, lhsT, inap,
                                             start=(ti == 0), stop=False)
                    for h in range(2):
                        nc.tensor.matmul(
                            pc[:, h, 0:NH],
                            cpeb[0:1, cv * C + cb * 128:cv * C + (cb + 1) * 128],
                            onesrow[0:1, h * NH:(h + 1) * NH],
                            start=False, stop=True)
                    # drain + residual add
                    nc.vector.scalar_tensor_tensor(
                        _h2(out[:, cb, :]), pc[:, :, 0:NH], 1.0,
                        _h2(res_src[:, cb, :].bitcast(F32)),
                        op0=ALU.mult, op1=ALU.add)
                return out

            def ln_stats(src, tag):
                """Returns (stag [2, N] f32r rows {ms, ones}; srep [128, N] f32r)."""
                sq = a1.tile([128, CB, N], BF16, tag="sq")
                for cb in range(CB):
                    nc.gpsimd.tensor_mul(sq[:, cb, :], src[:, cb, :].bitcast(F32),
                                         src[:, cb, :].bitcast(F32))
                psx = pw.tile([1, 2, 512], F32, tag="pw")
                psq = pw.tile([1, 2, 512], F32, tag="pw")
                for h in range(2):
                    hs = slice(h * NH, (h + 1) * NH)
                    for cb in range(CB):
                        nc.tensor.matmul(psx[0:1, h, 0:NH], onesp[:, 0:1],
                                         src[:, cb, hs], start=(cb == 0),
                                         stop=(cb == CB - 1))
                        nc.tensor.matmul(psq[0:1, h, 0:NH], onespb[:],
                                         sq[:, cb, hs], start=(cb == 0),
                                         stop=(cb == CB - 1))
                mu = a1.tile([1, N], F32, tag="mu")
                nc.vector.tensor_scalar(_h2(mu[:]), psx[0:1, :, 0:NH], 1.0 / C,
                                        None, op0=ALU.mult)
                e2 = a3.tile([1, N], F32, tag="lnsm")
                # E2 + eps fused here so Ln needs no bias const
                nc.vector.tensor_scalar(_h2(e2[:]), psq[0:1, :, 0:NH], 1.0 / C,
                                        EPS, op0=ALU.mult, op1=ALU.add)
                musq = a3.tile([1, N], F32, tag="lnsm")
                nc.gpsimd.tensor_mul(musq[:], mu[:], mu[:])
                var0 = a3.tile([1, N], F32, tag="lnsm")
                nc.vector.scalar_tensor_tensor(var0[:], musq[:], -1.0, e2[:],
                                               op0=ALU.mult, op1=ALU.add)
                lnv = a3.tile([1, N], F32, tag="lnsm")
                nc.scalar.activation(lnv[:], var0[:], AF.Ln)
                srow = a3.tile([1, N], F32R, tag="lnsm")
                nc.scalar.activation(srow[:], lnv[:], AF.Exp, scale=-0.5)
                stag = a1.tile([2, N], F32R, tag="stag")
                nc.vector.tensor_mul(stag[0:1, :], mu[:], srow[:].bitcast(F32))
                nc.sync.dma_start(stag[1:2, :], onesrow_d[:].bitcast(F32R))
                psr = pw.tile([128, 2, 512], F32, tag="pw")
                for h in range(2):
                    nc.tensor.matmul(psr[:, h, 0:NH], onesrow[0:1, 0:128],
                                     srow[0:1, h * NH:(h + 1) * NH],
                                     start=True, stop=True)
                srep = a1.tile([128, N], F32R, tag="srep")
                nc.scalar.activation(_h2(srep[:]), psr[:, :, 0:NH], AF.Copy)
                return stag, srep

            # ---------------- per-image pipeline ----------------
            for img in range(BLOC):
                x0 = a1.tile([128, CB, N], F32R, tag="x0")
                x0p = a1.tile([128, CB, PADW], BF16, tag="cvb")
                nc.gpsimd.memset(x0p[:], 0.0)
                for cb in range(CB):
                    nc.sync.dma_start(x0[:, cb, :],
                                      x_d[img, cb * 128:(cb + 1) * 128, :].bitcast(F32R))
                    nc.sync.dma_start(
                        x0p[:, cb, 1:813].rearrange("p (r c) -> p r c", c=29)[:, :, 0:HWD],
                        _rc(xb_d[img, cb * 128:(cb + 1) * 128, :]))

                y1 = conv_core(x0p, x0, 0, "y1")
                stats1, srep1 = ln_stats(y1, "st1")

                xs1 = a1.tile([128, CB, N], F32R, tag="scr")
                for cb in range(CB):
                    nc.vector.tensor_mul(xs1[:, cb, :], y1[:, cb, :].bitcast(F32),
                                         srep1[:].bitcast(F32))

                # ---- q (channel-major, bf16 out)
                q = a1.tile([128, CB, N], BF16, tag="q")
                for cb in range(CB):
                    pq = pw.tile([128, 2, 512], F32, tag="pw")
                    for h in range(2):
                        hs = slice(h * NH, (h + 1) * NH)
                        for kb in range(CB):
                            nc.tensor.matmul(pq[:, h, 0:NH],
                                             wqkv[:, kb, cb * 128:(cb + 1) * 128],
                                             xs1[:, kb, hs],
                                             start=(kb == 0), stop=False)
                        nc.tensor.matmul(pq[:, h, 0:NH],
                                         rkq[:, cb * 128:(cb + 1) * 128],
                                         stats1[0:2, hs], start=False, stop=True)
                    nc.scalar.activation(_h2(q[:, cb, :]), pq[:, :, 0:NH], AF.Copy)

                # ---- kT / vT (token-major, bf16 out)
                kT = a1.tile([128, 7, C], BF16, tag="kT")
                vT = a1.tile([128, 7, C], BF16, tag="vT")
                for ci, (t0, nt) in enumerate(CHUNKS):
                    for dst, wc0, rc0 in ((kT, C, 0), (vT, 2 * C, C)):
                        pkv = pn.tile([128, 512], F32, tag="pn")
                        for kb in range(CB):
                            nc.tensor.matmul(pkv[0:nt, 0:C],
                                             xs1[:, kb, t0:t0 + nt],
                                             wqkv[:, kb, wc0:wc0 + C],
                                             start=(kb == 0), stop=False)
                        nc.tensor.matmul(pkv[0:nt, 0:C],
                                         stats1[0:2, t0:t0 + nt],
                                         rkkv[:, rc0:rc0 + C],
                                         start=False, stop=True)
                        nc.vector.tensor_copy(dst[0:nt, ci, :], pkv[0:nt, 0:C])

                # ---- einsum1 both orientations + softmax pieces
                aed = a1.tile([128, CB, 128], BF16, tag="aed")
                recip = a1.tile([128, CB], F32, tag="recip")
                zacc = a1.tile([128, CB], F32, tag="zacc")
                for cb in range(CB):
                    cbs = slice(cb * 128, (cb + 1) * 128)
                    pde = pn.tile([128, 512], F32, tag="pn")
                    for ci, (t0, nt) in enumerate(CHUNKS):
                        nc.tensor.matmul(pde[:, 0:128], kT[0:nt, ci, cbs],
                                         vT[0:nt, ci, cbs],
                                         start=(ci == 0), stop=False)
                    nc.tensor.matmul(pde[:, 0:128], maskl[:], maskr[:],
                                     start=False, stop=True)
                    scrap = a1.tile([128, 128], BF16, tag="scrap")
                    nc.scalar.activation(scrap[:], pde[:, 0:128], AF.Exp,
                                         accum_out=zacc[:, cb:cb + 1])
                    nc.vector.reciprocal(recip[:, cb:cb + 1], zacc[:, cb:cb + 1])
                    ped = pn.tile([128, 512], F32, tag="pn")
                    for ci, (t0, nt) in enumerate(CHUNKS):
                        nc.tensor.matmul(ped[:, 0:128], vT[0:nt, ci, cbs],
                                         kT[0:nt, ci, cbs],
                                         start=(ci == 0), stop=(ci == 6))
                    nc.scalar.activation(aed[:, cb, :], ped[:, 0:128], AF.Exp)

                # ---- einsum2 -> attn (1/Z folded into drain)
                attn = a1.tile([128, CB, N], F32R, tag="scr")
                for cb in range(CB):
                    pe2 = pw.tile([128, 2, 512], F32, tag="pw")
                    for h in range(2):
                        hs = slice(h * NH, (h + 1) * NH)
                        for j in range(4):
                            js = slice(32 * j, 32 * j + 32)
                            nc.tensor.matmul(pe2[js, h, 0:NH],
                                             aed[js, cb, js],
                                             q[js, cb, hs],
                                             start=True, stop=True,
                                             tile_position=(32 * j, 32 * j))
                    nc.vector.tensor_scalar(_h2(attn[:, cb, :]), pe2[:, :, 0:NH],
                                            recip[:, cb:cb + 1], None, op0=ALU.mult)

                # ---- proj + bias + residual -> x2
                x2 = a1.tile([128, CB, N], F32R, tag="x2")
                for cb in range(CB):
                    pp = pw.tile([128, 2, 512], F32, tag="pw")
                    for h in range(2):
                        hs = slice(h * NH, (h + 1) * NH)
                        for kb in range(CB):
                            nc.tensor.matmul(pp[:, h, 0:NH],
                                             wproj[:, kb, cb * 128:(cb + 1) * 128],
                                             attn[:, kb, hs],
                                             start=(kb == 0), stop=(kb == CB - 1))
                    nc.vector.scalar_tensor_tensor(_h2(x2[:, cb, :]), pp[:, :, 0:NH],
                                                   pbias[:, cb:cb + 1],
                                                   _h2(y1[:, cb, :].bitcast(F32)),
                                                   op0=ALU.add, op1=ALU.add)

                # padded bf16 copy of x2 for conv2 taps
                x2p = a1.tile([128, CB, PADW], BF16, tag="cvb")
                nc.gpsimd.memset(x2p[:], 0.0)
                for cb in range(CB):
                    nc.gpsimd.tensor_copy(
                        x2p[:, cb, 1:813].rearrange("p (r c) -> p r c", c=29)[:, :, 0:HWD],
                        _rc(x2[:, cb, :].bitcast(F32)))

                y2 = conv_core(x2p, x2, 1, "y2")
                stats2, srep2 = ln_stats(y2, "st2")

                xs2 = a1.tile([128, CB, N], F32R, tag="scr")
                for cb in range(CB):
                    nc.vector.tensor_mul(xs2[:, cb, :], y2[:, cb, :].bitcast(F32),
                                         srep2[:].bitcast(F32))

                # ---- MLP (per token-half) + residual -> out
                outs = a1.tile([128, CB, N], F32, tag="outs")
                for h in range(2):
                    hs = slice(h * NH, (h + 1) * NH)
                    geluh = a1.tile([128, HB, NH], F32R, tag="gelu")
                    for j in range(HB):
                        pf = pn.tile([128, 512], F32, tag="pn")
                        for kb in range(CB):
                            nc.tensor.matmul(pf[:, 0:NH],
                                             wfc1[:, kb, j * 128:(j + 1) * 128],
                                             xs2[:, kb, hs],
                                             start=(kb == 0), stop=False)
                        nc.tensor.matmul(pf[:, 0:NH],
                                         rkf[:, j * 128:(j + 1) * 128],
                                         stats2[0:2, hs], start=False, stop=True)
                        nc.scalar.activation(geluh[:, j, :], pf[:, 0:NH], AF.Gelu)
                    for cb in range(CB):
                        p2 = pn.tile([128, 512], F32, tag="pn")
                        for j in range(HB):
                            nc.tensor.matmul(p2[:, 0:NH],
                                             wfc2[:, j, cb * 128:(cb + 1) * 128],
                                             geluh[:, j, :],
                                             start=(j == 0), stop=(j == HB - 1))
                        nc.vector.scalar_tensor_tensor(outs[:, cb, hs], p2[:, 0:NH],
                                                       pbias[:, 3 + cb:4 + cb],
                                                       y2[:, cb, hs].bitcast(F32),
                                                       op0=ALU.add, op1=ALU.add)
                for cb in range(CB):
                    nc.sync.dma_start(out_d[img, cb * 128:(cb + 1) * 128, :],
                                      outs[:, cb, :])

    if legalize:
        _legalize_multiwaits(nc)
    return nc


def _host_consts(cpe1_w, cpe1_b, norm1_g, norm1_b, qkv_w, proj_w, proj_b,
                 cpe2_w, cpe2_b, norm2_g, norm2_b, fc1_w, fc1_b, fc2_w, fc2_b):
    f = np.float32
    qkv_f = (qkv_w * norm1_g[None, :]).astype(f)
    qkv_f[C:2 * C] *= SCALE
    bias_qkv = (qkv_w @ norm1_b).astype(f)
    bias_qkv[C:2 * C] *= SCALE
    rkq = np.stack([-qkv_f[0:C].sum(1), bias_qkv[0:C]]).astype(f)
    rkkv = np.concatenate(
        [np.stack([-qkv_f[C:2 * C].sum(1), bias_qkv[C:2 * C]]),
         np.stack([-qkv_f[2 * C:].sum(1), bias_qkv[2 * C:]])], axis=1).astype(f)
    fc1_f = (fc1_w * norm2_g[None, :]).astype(f)
    rkf = np.stack([-fc1_f.sum(1), (fc1_b + fc1_w @ norm2_b)]).astype(f)

    diag = np.zeros((2, 9, CB, 128, 128), f)
    for cv, w in enumerate((cpe1_w, cpe2_w)):
        w9 = w.reshape(C, 9)
        for ti, (dy, dx) in enumerate(TAPS):
            tf = (dy + 1) * 3 + (dx + 1)
            for cb in range(CB):
                d = w9[cb * 128:(cb + 1) * 128, tf]
                diag[cv, ti, cb][np.arange(128), np.arange(128)] = d

    maskl = np.zeros((5, 128), f)
    maskr = np.zeros((5, 128), f)
    maskl[0] = 1.0
    maskr[0] = -30.0
    for j in range(4):
        maskl[1 + j, 32 * j:32 * j + 32] = 1.0
        maskr[1 + j, 32 * j:32 * j + 32] = 30.0

    pbias = np.zeros((128, 6), f)
    pbias[:, 0:3] = proj_b.reshape(CB, 128).T
    pbias[:, 3:6] = fc2_b.reshape(CB, 128).T

    return {
        "wqkv": np.ascontiguousarray(qkv_f.T),
        "wproj": np.ascontiguousarray(proj_w.astype(f).T),
        "wfc1": np.ascontiguousarray(fc1_f.T),
        "wfc2": np.ascontiguousarray(fc2_w.astype(f).T),
        "diag": diag.astype(ml_dtypes.bfloat16),
        "rkq": rkq,
        "rkkv": rkkv,
        "rkf": rkf,
        "cpeb": np.concatenate([cpe1_b, cpe2_b]).reshape(1, 2 * C).astype(f),
        "onesrow": np.ones((1, N), f),
        "onesp": np.ones((128, 2), f),
        "onespb": np.ones((128, 1), ml_dtypes.bfloat16),
        "pbias": pbias,
        "maskl": maskl.astype(ml_dtypes.bfloat16),
        "maskr": maskr.astype(ml_dtypes.bfloat16),
    }


_PROG = None


def kernel(**inputs):
    global _PROG
    from concourse.bass_utils import run_bass_kernel_spmd

    x = np.asarray(inputs["x"], dtype=np.float32)
    B = x.shape[0]
    consts = _host_consts(
        np.asarray(inputs["cpe1_w"], np.float32),
        np.asarray(inputs["cpe1_b"], np.float32),
        np.asarray(inputs["norm1_g"], np.float32),
        np.asarray(inputs["norm1_b"], np.float32),
        np.asarray(inputs["qkv_w"], np.float32),
        np.asarray(inputs["proj_w"], np.float32),
        np.asarray(inputs["proj_b"], np.float32),
        np.asarray(inputs["cpe2_w"], np.float32),
        np.asarray(inputs["cpe2_b"], np.float32),
        np.asarray(inputs["norm2_g"], np.float32),
        np.asarray(inputs["norm2_b"], np.float32),
        np.asarray(inputs["fc1_w"], np.float32),
        np.asarray(inputs["fc1_b"], np.float32),
        np.asarray(inputs["fc2_w"], np.float32),
        np.asarray(inputs["fc2_b"], np.float32),
    )

    if _PROG is None:
        _PROG = build_program()
    nc = _PROG

    xr = np.ascontiguousarray(x.reshape(B, C, N))
    xbr = xr.astype(ml_dtypes.bfloat16)
    in_maps = []
    for core in range(NCORES):
        m = dict(consts)
        m["x"] = np.ascontiguousarray(xr[core * BLOC:(core + 1) * BLOC])
        m["xb"] = np.ascontiguousarray(xbr[core * BLOC:(core + 1) * BLOC])
        in_maps.append(m)

    trace = os.environ.get("CCK_TRACE", "0") == "1"
    res = run_bass_kernel_spmd(nc, in_maps, core_ids=list(range(NCORES)),
                               trace=trace)
    globals()["LAST_RESULTS"] = res
    out = np.concatenate([r["out"] for r in res.results], axis=0)
    return out.reshape(B, C, HWD, HWD).astype(np.float32)


# revision 13
# speedup vs baseline: 1.4093x; 1.4093x over previous
"""Trainium2 Bass kernel for nn_ChannelBlock (XCiT-style channel-attention block).

Sharding: data-parallel over batch (32 images -> 8 cores x 4 images). Each core
runs the full block on its 4 images. Matmuls run in fp32r (full-rate, ~11
mantissa bits input rounding, fp32 accumulate); the tiny attention einsums and
the depthwise-conv taps run in bf16 (their contribution to the output is small).

Structure per image (channel-major [C_part, N_free] activations):
  conv1   9 bf16 diagonal-matmul taps w/ shifted boundary-clipped APs + f32r
          bias tap into PSUM; residual added in the PSUM drain (stt on DVE).
  LN1     sums via ones-matmuls into PSUM; 1/sqrt(var+eps) = exp(-.5*ln()) on
          ACT; s replicated across partitions by a K=1 ones matmul; gain g
          folded into the qkv weights host-side; mean/bias applied as K=2
          rank corrections {ms=mu*s, ones} x {-rowsum(W'), W@b} in PSUM.
  attn    q channel-major; kT/vT token-major (so k^T v contracts on the
          partition dim), both bf16; einsum1 computed in both orientations
          ([d,e] for row sums via exp's fused accum_out + additive -30 mask
          off the head-diagonal; [e,d] as einsum2's stationary operand);
          softmax 1/Z folded into the einsum2 PSUM drain.
  proj    f32r matmul; bias+residual in the drain (stt).
  conv2 / LN2 / MLP (fc1+exact gelu+fc2) same tricks; fc2 drain adds bias and
          the residual, then DMA out.
"""
import os
import sys
sys.path.insert(0, "/opt/trn_rl_repo")

import numpy as np
import ml_dtypes

import concourse.bass as bass
import concourse.mybir as mybir
import concourse.tile as tile

F32 = mybir.dt.float32
F32R = mybir.dt.float32r
BF16 = mybir.dt.bfloat16
AF = mybir.ActivationFunctionType
ALU = mybir.AluOpType

NCORES = 8
BLOC = 4
C = 384
CB = 3
HWD = 28
N = 784
NH = 392
HEADS = 12
HD = 32
HIDDEN = 1536
HB = 12
SCALE = HD ** -0.5
EPS = 1e-5
CHUNKS = [(0, 128), (128, 128), (256, 128), (384, 128),
          (512, 128), (640, 128), (768, 16)]
TAPS = [(0, 0)] + [(dy, dx) for dy in (-1, 0, 1) for dx in (-1, 0, 1)
                   if (dy, dx) != (0, 0)]
PADW = 816  # pitch-29 padded conv-input row layout: 1 leading pad + 28x29 + tail


def _legalize_multiwaits(nc):
    """This walrus build accepts at most ONE semaphore wait per instruction.
    Tile keeps almost everything at <=1 but the kernel-tail drain (and the
    fused fp32r matmuls) can carry more; splice NoOp carriers in front."""
    n = 0
    for fn in nc.m.functions:
        for blk in fn.blocks:
            new_list = []
            changed = False
            for inst in blk.instructions:
                si = inst.sync_info
                if si is not None and si.on_wait is not None and len(si.on_wait) > 1:
                    waits = list(si.on_wait)
                    for i, w in enumerate(waits[:-1]):
                        new_list.append(mybir.InstNoOp(
                            name=f"{inst.name}_waitcarrier{i}",
                            opcode="NoOp",
                            engine=inst.engine,
                            sync_info=mybir.SyncInfo(on_wait=[w], on_update=[]),
                        ))
                        n += 1
                    si.on_wait = waits[-1:]
                    changed = True
                new_list.append(inst)
            if changed:
                blk.instructions[:] = new_list
    return n


def _h2(ap):
    return ap.rearrange("p (h n) -> p h n", h=2)


def _h2rc(ap):
    return ap.rearrange("p (h r c) -> p h r c", h=2, c=HWD)


def _rc(ap):
    return ap.rearrange("p (r c) -> p r c", c=HWD)


def build_program(legalize=True):
    nc = bass.Bass("TRN2", target_bir_lowering=False)

    x_d = nc.dram_tensor("x", [BLOC, C, N], F32, kind="ExternalInput")
    xb_d = nc.dram_tensor("xb", [BLOC, C, N], BF16, kind="ExternalInput")
    wqkv_d = nc.dram_tensor("wqkv", [C, 3 * C], BF16, kind="ExternalInput")
    wproj_d = nc.dram_tensor("wproj", [C, C], BF16, kind="ExternalInput")
    wfc1_d = nc.dram_tensor("wfc1", [C, HIDDEN], BF16, kind="ExternalInput")
    wfc2_d = nc.dram_tensor("wfc2", [HIDDEN, C], BF16, kind="ExternalInput")
    diag_d = nc.dram_tensor("diag", [2, 9, CB, 128, 128], BF16, kind="ExternalInput")
    rkq_d = nc.dram_tensor("rkq", [2, C], BF16, kind="ExternalInput")
    rkkv_d = nc.dram_tensor("rkkv", [2, 2 * C], BF16, kind="ExternalInput")
    rkf_d = nc.dram_tensor("rkf", [2, HIDDEN], BF16, kind="ExternalInput")
    cpeb_d = nc.dram_tensor("cpeb", [1, 2 * C], F32, kind="ExternalInput")
    onesrow_d = nc.dram_tensor("onesrow", [1, N], F32, kind="ExternalInput")
    onesrowb_d = nc.dram_tensor("onesrowb", [1, N], BF16, kind="ExternalInput")
    onesp_d = nc.dram_tensor("onesp", [128, 2], F32, kind="ExternalInput")
    onespb_d = nc.dram_tensor("onespb", [128, 1], BF16, kind="ExternalInput")
    pbias_d = nc.dram_tensor("pbias", [128, 6], F32, kind="ExternalInput")
    maskl_d = nc.dram_tensor("maskl", [5, 128], BF16, kind="ExternalInput")
    maskr_d = nc.dram_tensor("maskr", [5, 128], BF16, kind="ExternalInput")
    out_d = nc.dram_tensor("out", [BLOC, C, N], F32, kind="ExternalOutput")

    with tile.TileContext(nc) as tc:
        with tc.tile_pool(name="const", bufs=1) as cp, \
             tc.tile_pool(name="act1", bufs=1) as a1, \
             tc.tile_pool(name="act3", bufs=3) as a3, \
             tc.tile_pool(name="psw", bufs=2, space="PSUM") as pw, \
             tc.tile_pool(name="psn", bufs=4, space="PSUM") as pn:

            # ---------------- constants ----------------
            wqkv = cp.tile([128, CB, 3 * C], BF16, tag="wqkv")
            wproj = cp.tile([128, CB, C], BF16, tag="wproj")
            wfc1 = cp.tile([128, CB, HIDDEN], BF16, tag="wfc1")
            wfc2 = cp.tile([128, HB, C], BF16, tag="wfc2")
            for cb in range(CB):
                nc.sync.dma_start(wqkv[:, cb, :],
                                  wqkv_d[cb * 128:(cb + 1) * 128, :])
                nc.sync.dma_start(wproj[:, cb, :],
                                  wproj_d[cb * 128:(cb + 1) * 128, :])
                nc.sync.dma_start(wfc1[:, cb, :],
                                  wfc1_d[cb * 128:(cb + 1) * 128, :])
            for j in range(HB):
                nc.sync.dma_start(wfc2[:, j, :],
                                  wfc2_d[j * 128:(j + 1) * 128, :])
            diag = cp.tile([128, 2, 9, CB, 128], BF16, tag="diag")
            for cv in range(2):
                for cb in range(CB):
                    nc.sync.dma_start(
                        diag[:, cv, :, cb, :],
                        diag_d[cv, :, cb, :, :].rearrange("t p f -> p t f"))
            rkq = cp.tile([2, C], BF16, tag="rkq")
            nc.sync.dma_start(rkq[:], rkq_d[:])
            rkkv = cp.tile([2, 2 * C], BF16, tag="rkkv")
            nc.sync.dma_start(rkkv[:], rkkv_d[:])
            rkf = cp.tile([2, HIDDEN], BF16, tag="rkf")
            nc.sync.dma_start(rkf[:], rkf_d[:])
            cpeb = cp.tile([1, 2 * C], F32R, tag="cpeb")
            nc.sync.dma_start(cpeb[:], cpeb_d[:].bitcast(F32R))
            onesrow = cp.tile([1, N], F32R, tag="onesrow")
            nc.sync.dma_start(onesrow[:], onesrow_d[:].bitcast(F32R))
            onesp = cp.tile([128, 2], F32R, tag="onesp")
            nc.sync.dma_start(onesp[:], onesp_d[:].bitcast(F32R))
            onespb = cp.tile([128, 1], BF16, tag="onespb")
            nc.sync.dma_start(onespb[:], onespb_d[:])
            pbias = cp.tile([128, 6], F32, tag="pbias")
            nc.sync.dma_start(pbias[:], pbias_d[:])
            maskl = cp.tile([5, 128], BF16, tag="maskl")
            nc.sync.dma_start(maskl[:], maskl_d[:])
            maskr = cp.tile([5, 128], BF16, tag="maskr")
            nc.sync.dma_start(maskr[:], maskr_d[:])

            # ---------------- helpers ----------------
            def conv_core(srcp, res_src, cv, tag):
                """srcp: bf16 [128, CB, PADW] pitch-29 padded tap source (1
                leading pad element, 1 pad col per row, all pads zero);
                res_src: f32r residual source.
                Returns out = res_src + dwconv(src) + b, f32r.

                Every tap writes FULL-WIDTH flat rows into PSUM; horizontal
                shifts read the zero pad column at row wraps, vertical edge
                rows are clipped."""
                out = a1.tile([128, CB, N], F32R, tag=tag)
                for cb in range(CB):
                    pc = pw.tile([128, 2, 512], F32, tag="pw")
                    for ti, (dy, dx) in enumerate(TAPS):
                        lhsT = diag[:, cv, ti, cb, :]
                        r0 = max(0, -dy)
                        r1 = HWD - max(0, dy)
                        for h in range(2):
                            rA = max(r0, 14 * h)
                            rB = min(r1, 14 * h + 14)
                            if rA >= rB:
                                continue
                            rl0, rl1 = rA - 14 * h, rB - 14 * h
                            outap = pc[:, h, rl0 * HWD:rl1 * HWD]
                            s0 = 1 + (rA + dy) * 29 + dx
                            inap = srcp[:, cb, s0:s0 + (rB - rA) * 29].rearrange(
                                "p (r c) -> p r c", c=29)[:, :, 0:HWD]
                            nc.tensor.matmul(outap, lhsT, inap,
                                             start=(ti == 0), stop=False)
                    for h in range(2):
                        nc.tensor.matmul(
                            pc[:, h, 0:NH],
                            cpeb[0:1, cv * C + cb * 128:cv * C + (cb + 1) * 128],
                            onesrow[0:1, h * NH:(h + 1) * NH],
                            start=False, stop=True)
                    # drain + residual add
                    nc.vector.scalar_tensor_tensor(
                        _h2(out[:, cb, :]), pc[:, :, 0:NH], 1.0,
                        _h2(res_src[:, cb, :].bitcast(F32)),
                        op0=ALU.mult, op1=ALU.add)
                return out

            def ln_stats(src, tag):
                """Returns (stag [2, N] f32r rows {ms, ones}; srep [128, N] f32r)."""
                sq = a1.tile([128, CB, N], BF16, tag="sq")
                for cb in range(CB):
                    nc.gpsimd.tensor_mul(sq[:, cb, :], src[:, cb, :].bitcast(F32),
                                         src[:, cb, :].bitcast(F32))
                psx = pw.tile([1, 2, 512], F32, tag="pw")
                psq = pw.tile([1, 2, 512], F32, tag="pw")
                for h in range(2):
                    hs = slice(h * NH, (h + 1) * NH)
                    for cb in range(CB):
                        nc.tensor.matmul(psx[0:1, h, 0:NH], onesp[:, 0:1],
                                         src[:, cb, hs], start=(cb == 0),
                                         stop=(cb == CB - 1))
                        nc.tensor.matmul(psq[0:1, h, 0:NH], onespb[:],
                                         sq[:, cb, hs], start=(cb == 0),
                                         stop=(cb == CB - 1))
                mu = a1.tile([1, N], F32, tag="mu")
                nc.vector.tensor_scalar(_h2(mu[:]), psx[0:1, :, 0:NH], 1.0 / C,
                                        None, op0=ALU.mult)
                e2 = a3.tile([1, N], F32, tag="lnsm")
                # E2 + eps fused here so Ln needs no bias const
                nc.vector.tensor_scalar(_h2(e2[:]), psq[0:1, :, 0:NH], 1.0 / C,
                                        EPS, op0=ALU.mult, op1=ALU.add)
                musq = a3.tile([1, N], F32, tag="lnsm")
                nc.gpsimd.tensor_mul(musq[:], mu[:], mu[:])
                var0 = a3.tile([1, N], F32, tag="lnsm")
                nc.vector.scalar_tensor_tensor(var0[:], musq[:], -1.0, e2[:],
                                               op0=ALU.mult, op1=ALU.add)
                lnv = a3.tile([1, N], F32, tag="lnsm")
                nc.scalar.activation(lnv[:], var0[:], AF.Ln)
                srow = a3.tile([1, N], F32R, tag="lnsm")
                nc.scalar.activation(srow[:], lnv[:], AF.Exp, scale=-0.5)
                stag = a1.tile([2, N], BF16, tag="stag")
                nc.vector.tensor_mul(stag[0:1, :], mu[:], srow[:].bitcast(F32))
                nc.sync.dma_start(stag[1:2, :], onesrowb_d[:])
                psr = pw.tile([128, 2, 512], F32, tag="pw")
                for h in range(2):
                    nc.tensor.matmul(psr[:, h, 0:NH], onesrow[0:1, 0:128],
                                     srow[0:1, h * NH:(h + 1) * NH],
                                     start=True, stop=True)
                srep = a1.tile([128, N], F32R, tag="srep")
                nc.scalar.activation(_h2(srep[:]), psr[:, :, 0:NH], AF.Copy)
                return stag, srep

            # ---------------- per-image pipeline ----------------
            for img in range(BLOC):
                x0 = a1.tile([128, CB, N], F32R, tag="x0")
                x0p = a1.tile([128, CB, PADW], BF16, tag="cvb")
                nc.gpsimd.memset(x0p[:], 0.0)
                for cb in range(CB):
                    nc.sync.dma_start(x0[:, cb, :],
                                      x_d[img, cb * 128:(cb + 1) * 128, :].bitcast(F32R))
                    nc.sync.dma_start(
                        x0p[:, cb, 1:813].rearrange("p (r c) -> p r c", c=29)[:, :, 0:HWD],
                        _rc(xb_d[img, cb * 128:(cb + 1) * 128, :]))

                y1 = conv_core(x0p, x0, 0, "y1")
                stats1, srep1 = ln_stats(y1, "st1")

                xs1 = a1.tile([128, CB, N], BF16, tag="scr")
                for cb in range(CB):
                    nc.vector.tensor_mul(xs1[:, cb, :], y1[:, cb, :].bitcast(F32),
                                         srep1[:].bitcast(F32))

                # ---- q (channel-major, bf16 out)
                q = a1.tile([128, CB, N], BF16, tag="q")
                for cb in range(CB):
                    pq = pw.tile([128, 2, 512], F32, tag="pw")
                    for h in range(2):
                        hs = slice(h * NH, (h + 1) * NH)
                        for kb in range(CB):
                            nc.tensor.matmul(pq[:, h, 0:NH],
                                             wqkv[:, kb, cb * 128:(cb + 1) * 128],
                                             xs1[:, kb, hs],
                                             start=(kb == 0), stop=False)
                        nc.tensor.matmul(pq[:, h, 0:NH],
                                         rkq[:, cb * 128:(cb + 1) * 128],
                                         stats1[0:2, hs], start=False, stop=True)
                    nc.scalar.activation(_h2(q[:, cb, :]), pq[:, :, 0:NH], AF.Copy)

                # ---- kT / vT (token-major, bf16 out)
                kT = a1.tile([128, 7, C], BF16, tag="kT")
                vT = a1.tile([128, 7, C], BF16, tag="vT")
                for ci, (t0, nt) in enumerate(CHUNKS):
                    for dst, wc0, rc0 in ((kT, C, 0), (vT, 2 * C, C)):
                        pkv = pn.tile([128, 512], F32, tag="pn")
                        for kb in range(CB):
                            nc.tensor.matmul(pkv[0:nt, 0:C],
                                             xs1[:, kb, t0:t0 + nt],
                                             wqkv[:, kb, wc0:wc0 + C],
                                             start=(kb == 0), stop=False)
                        nc.tensor.matmul(pkv[0:nt, 0:C],
                                         stats1[0:2, t0:t0 + nt],
                                         rkkv[:, rc0:rc0 + C],
                                         start=False, stop=True)
                        nc.vector.tensor_copy(dst[0:nt, ci, :], pkv[0:nt, 0:C])

                # ---- einsum1 both orientations + softmax pieces
                aed = a1.tile([128, CB, 128], BF16, tag="aed")
                recip = a1.tile([128, CB], F32, tag="recip")
                zacc = a1.tile([128, CB], F32, tag="zacc")
                for cb in range(CB):
                    cbs = slice(cb * 128, (cb + 1) * 128)
                    pde = pn.tile([128, 512], F32, tag="pn")
                    for ci, (t0, nt) in enumerate(CHUNKS):
                        nc.tensor.matmul(pde[:, 0:128], kT[0:nt, ci, cbs],
                                         vT[0:nt, ci, cbs],
                                         start=(ci == 0), stop=False)
                    nc.tensor.matmul(pde[:, 0:128], maskl[:], maskr[:],
                                     start=False, stop=True)
                    scrap = a1.tile([128, 128], BF16, tag="scrap")
                    nc.scalar.activation(scrap[:], pde[:, 0:128], AF.Exp,
                                         accum_out=zacc[:, cb:cb + 1])
                    nc.vector.reciprocal(recip[:, cb:cb + 1], zacc[:, cb:cb + 1])
                    ped = pn.tile([128, 512], F32, tag="pn")
                    for ci, (t0, nt) in enumerate(CHUNKS):
                        nc.tensor.matmul(ped[:, 0:128], vT[0:nt, ci, cbs],
                                         kT[0:nt, ci, cbs],
                                         start=(ci == 0), stop=(ci == 6))
                    nc.scalar.activation(aed[:, cb, :], ped[:, 0:128], AF.Exp)

                # ---- einsum2 -> attn (1/Z folded into drain)
                attn = a1.tile([128, CB, N], BF16, tag="scr")
                for cb in range(CB):
                    pe2 = pw.tile([128, 2, 512], F32, tag="pw")
                    for h in range(2):
                        hs = slice(h * NH, (h + 1) * NH)
                        for j in range(4):
                            js = slice(32 * j, 32 * j + 32)
                            nc.tensor.matmul(pe2[js, h, 0:NH],
                                             aed[js, cb, js],
                                             q[js, cb, hs],
                                             start=True, stop=True,
                                             tile_position=(32 * j, 32 * j))
                    nc.vector.tensor_scalar(_h2(attn[:, cb, :]), pe2[:, :, 0:NH],
                                            recip[:, cb:cb + 1], None, op0=ALU.mult)

                # ---- proj + bias + residual -> x2
                x2 = a1.tile([128, CB, N], F32R, tag="x2")
                for cb in range(CB):
                    pp = pw.tile([128, 2, 512], F32, tag="pw")
                    for h in range(2):
                        hs = slice(h * NH, (h + 1) * NH)
                        for kb in range(CB):
                            nc.tensor.matmul(pp[:, h, 0:NH],
                                             wproj[:, kb, cb * 128:(cb + 1) * 128],
                                             attn[:, kb, hs],
                                             start=(kb == 0), stop=(kb == CB - 1))
                    nc.vector.scalar_tensor_tensor(_h2(x2[:, cb, :]), pp[:, :, 0:NH],
                                                   pbias[:, cb:cb + 1],
                                                   _h2(y1[:, cb, :].bitcast(F32)),
                                                   op0=ALU.add, op1=ALU.add)

                # padded bf16 copy of x2 for conv2 taps
                x2p = a1.tile([128, CB, PADW], BF16, tag="cvb")
                nc.gpsimd.memset(x2p[:], 0.0)
                for cb in range(CB):
                    nc.gpsimd.tensor_copy(
                        x2p[:, cb, 1:813].rearrange("p (r c) -> p r c", c=29)[:, :, 0:HWD],
                        _rc(x2[:, cb, :].bitcast(F32)))

                y2 = conv_core(x2p, x2, 1, "y2")
                stats2, srep2 = ln_stats(y2, "st2")

                xs2 = a1.tile([128, CB, N], BF16, tag="scr")
                for cb in range(CB):
                    nc.vector.tensor_mul(xs2[:, cb, :], y2[:, cb, :].bitcast(F32),
                                         srep2[:].bitcast(F32))

                # ---- MLP (per token-half) + residual -> out
                outs = a1.tile([128, CB, N], F32, tag="outs")
                for h in range(2):
                    hs = slice(h * NH, (h + 1) * NH)
                    geluh = a1.tile([128, HB, NH], BF16, tag="gelu")
                    for j in range(HB):
                        pf = pn.tile([128, 512], F32, tag="pn")
                        for kb in range(CB):
                            nc.tensor.matmul(pf[:, 0:NH],
                                             wfc1[:, kb, j * 128:(j + 1) * 128],
                                             xs2[:, kb, hs],
                                             start=(kb == 0), stop=False)
                        nc.tensor.matmul(pf[:, 0:NH],
                                         rkf[:, j * 128:(j + 1) * 128],
                                         stats2[0:2, hs], start=False, stop=True)
                        nc.scalar.activation(geluh[:, j, :], pf[:, 0:NH], AF.Gelu)
                    for cb in range(CB):
                        p2 = pn.tile([128, 512], F32, tag="pn")
                        for j in range(HB):
                            nc.tensor.matmul(p2[:, 0:NH],
                                             wfc2[:, j, cb * 128:(cb + 1) * 128],
                                             geluh[:, j, :],
                                             start=(j == 0), stop=(j == HB - 1))
                        nc.vector.scalar_tensor_tensor(outs[:, cb, hs], p2[:, 0:NH],
                                                       pbias[:, 3 + cb:4 + cb],
                                                       y2[:, cb, hs].bitcast(F32),
                                                       op0=ALU.add, op1=ALU.add)
                for cb in range(CB):
                    nc.sync.dma_start(out_d[img, cb * 128:(cb + 1) * 128, :],
                                      outs[:, cb, :])

    if legalize:
        _legalize_multiwaits(nc)
    return nc


def _host_consts(cpe1_w, cpe1_b, norm1_g, norm1_b, qkv_w, proj_w, proj_b,
                 cpe2_w, cpe2_b, norm2_g, norm2_b, fc1_w, fc1_b, fc2_w, fc2_b):
    f = np.float32
    qkv_f = (qkv_w * norm1_g[None, :]).astype(f)
    qkv_f[C:2 * C] *= SCALE
    bias_qkv = (qkv_w @ norm1_b).astype(f)
    bias_qkv[C:2 * C] *= SCALE
    rkq = np.stack([-qkv_f[0:C].sum(1), bias_qkv[0:C]]).astype(f)
    rkkv = np.concatenate(
        [np.stack([-qkv_f[C:2 * C].sum(1), bias_qkv[C:2 * C]]),
         np.stack([-qkv_f[2 * C:].sum(1), bias_qkv[2 * C:]])], axis=1).astype(f)
    fc1_f = (fc1_w * norm2_g[None, :]).astype(f)
    rkf = np.stack([-fc1_f.sum(1), (fc1_b + fc1_w @ norm2_b)]).astype(f)

    diag = np.zeros((2, 9, CB, 128, 128), f)
    for cv, w in enumerate((cpe1_w, cpe2_w)):
        w9 = w.reshape(C, 9)
        for ti, (dy, dx) in enumerate(TAPS):
            tf = (dy + 1) * 3 + (dx + 1)
            for cb in range(CB):
                d = w9[cb * 128:(cb + 1) * 128, tf]
                diag[cv, ti, cb][np.arange(128), np.arange(128)] = d

    maskl = np.zeros((5, 128), f)
    maskr = np.zeros((5, 128), f)
    maskl[0] = 1.0
    maskr[0] = -30.0
    for j in range(4):
        maskl[1 + j, 32 * j:32 * j + 32] = 1.0
        maskr[1 + j, 32 * j:32 * j + 32] = 30.0

    pbias = np.zeros((128, 6), f)
    pbias[:, 0:3] = proj_b.reshape(CB, 128).T
    pbias[:, 3:6] = fc2_b.reshape(CB, 128).T

    return {
        "wqkv": np.ascontiguousarray(qkv_f.T).astype(ml_dtypes.bfloat16),
        "wproj": np.ascontiguousarray(proj_w.astype(f).T).astype(ml_dtypes.bfloat16),
        "wfc1": np.ascontiguousarray(fc1_f.T).astype(ml_dtypes.bfloat16),
        "wfc2": np.ascontiguousarray(fc2_w.astype(f).T).astype(ml_dtypes.bfloat16),
        "diag": diag.astype(ml_dtypes.bfloat16),
        "rkq": rkq.astype(ml_dtypes.bfloat16),
        "rkkv": rkkv.astype(ml_dtypes.bfloat16),
        "rkf": rkf.astype(ml_dtypes.bfloat16),
        "cpeb": np.concatenate([cpe1_b, cpe2_b]).reshape(1, 2 * C).astype(f),
        "onesrow": np.ones((1, N), f),
        "onesrowb": np.ones((1, N), ml_dtypes.bfloat16),
        "onesp": np.ones((128, 2), f),
        "onespb": np.ones((128, 1), ml_dtypes.bfloat16),
        "pbias": pbias,
        "maskl": maskl.astype(ml_dtypes.bfloat16),
        "maskr": maskr.astype(ml_dtypes.bfloat16),
    }


_PROG = None


def kernel(**inputs):
    global _PROG
    from concourse.bass_utils import run_bass_kernel_spmd

    x = np.asarray(inputs["x"], dtype=np.float32)
    B = x.shape[0]
    consts = _host_consts(
        np.asarray(inputs["cpe1_w"], np.float32),
        np.asarray(inputs["cpe1_b"], np.float32),
        np.asarray(inputs["norm1_g"], np.float32),
        np.asarray(inputs["norm1_b"], np.float32),
        np.asarray(inputs["qkv_w"], np.float32),
        np.asarray(inputs["proj_w"], np.float32),
        np.asarray(inputs["proj_b"], np.float32),
        np.asarray(inputs["cpe2_w"], np.float32),
        np.asarray(inputs["cpe2_b"], np.float32),
        np.asarray(inputs["norm2_g"], np.float32),
        np.asarray(inputs["norm2_b"], np.float32),
        np.asarray(inputs["fc1_w"], np.float32),
        np.asarray(inputs["fc1_b"], np.float32),
        np.asarray(inputs["fc2_w"], np.float32),
        np.asarray(inputs["fc2_b"], np.float32),
    )

    if _PROG is None:
        _PROG = build_program()
    nc = _PROG

    xr = np.ascontiguousarray(x.reshape(B, C, N))
    xbr = xr.astype(ml_dtypes.bfloat16)
    in_maps = []
    for core in range(NCORES):
        m = dict(consts)
        m["x"] = np.ascontiguousarray(xr[core * BLOC:(core + 1) * BLOC])
        m["xb"] = np.ascontiguousarray(xbr[core * BLOC:(core + 1) * BLOC])
        in_maps.append(m)

    trace = os.environ.get("CCK_TRACE", "0") == "1"
    res = run_bass_kernel_spmd(nc, in_maps, core_ids=list(range(NCORES)),
                               trace=trace)
    globals()["LAST_RESULTS"] = res
    out = np.concatenate([r["out"] for r in res.results], axis=0)
    return out.reshape(B, C, HWD, HWD).astype(np.float32)
